# revision 3
# baseline (speedup 1.0000x reference)
"""Single-head causal attention (B=8, T=4096, D=1024, H=64) on 8 TRN2 cores.

Strategy: data-parallel over batch — core b computes attention for x[b]
independently; no collectives. Per core:

  - Host pre-transposes x[b] -> xT [D, T] fp16 (layout choice; all FLOPs
    stay on device) and passes W_q/W_k/W_v as [D, H] fp16.
  - QK projection as ONE fused M=128 matmul per (d-chunk, n-tile):
    lhsT = [WqT_d | WkT_d] -> psum [qT; kT].
  - q2T/k2T [128, T]: duplicated-row layout (q on partitions 0-63 AND
    64-127) produced via a "duplication matrix" matmul, enabling 2-way
    row-packed S^T matmuls (K=64 each, two concurrent via PE row groups).
  - S^T computed tk-on-partitions: S^T[jblock, g] = kT_j.T @ qT_g, so the
    softmax transpose is never needed: P^T = exp(0.125*S^T - 4) straight
    from PSUM via ScalarE into fp16 SBUF. Fixed bias -4 replaces the
    per-row max (scores are bounded ~12; exact cancellation in the final
    normalization).
  - Causal mask applied post-exp as a 0/1 fp16 multiply on the 4 diagonal
    j-blocks of each q-group only.
  - PV: OT_aug[65, 512] += V_aug_j.T-style matmul with V augmented by a
    ones column, so row 64 accumulates the softmax denominator for free.
  - Output is OT_aug [65, T] f32; host divides by row 64 and transposes.
"""

import os
import sys

sys.path.insert(0, "/opt/trn_rl_repo")

import numpy as np

T, D, H = 4096, 1024, 64
NCORES = 8
ND = D // 128   # 8 contraction chunks
NT = T // 512   # 8 n-tiles / q-groups
NJ = T // 128   # 32 k-blocks

_cache = {}


def _build():
    if "nc" in _cache:
        return _cache["nc"]
    from contextlib import ExitStack

    import concourse.mybir as mybir
    import concourse.tile as tile
    from concourse import bacc

    dt = mybir.dt
    f16, f32 = dt.float16, dt.float32
    AF = mybir.ActivationFunctionType
    ALU = mybir.AluOpType

    nc = bacc.Bacc("TRN2", target_bir_lowering=False, debug=False)

    xt_h = nc.declare_dram_parameter("xt", [D, T], f16, isOutput=False)
    wq_h = nc.declare_dram_parameter("wq", [D, H], f16, isOutput=False)
    wk_h = nc.declare_dram_parameter("wk", [D, H], f16, isOutput=False)
    wv_h = nc.declare_dram_parameter("wv", [D, H], f16, isOutput=False)
    dup_h = nc.declare_dram_parameter("dup", [128, 128], f16, isOutput=False)
    idn_h = nc.declare_dram_parameter("idn", [64, 64], f16, isOutput=False)
    out_h = nc.declare_dram_parameter("out", [H + 1, T], f32, isOutput=True)

    with tile.TileContext(nc) as tc, ExitStack() as ctx:
        const = ctx.enter_context(tc.tile_pool(name="const", bufs=1))
        xpool = ctx.enter_context(tc.tile_pool(name="x", bufs=1))
        proj = ctx.enter_context(tc.tile_pool(name="proj", bufs=1))
        qkpool = ctx.enter_context(tc.tile_pool(name="qksb", bufs=3))
        ptpool = ctx.enter_context(tc.tile_pool(name="pt", bufs=4))
        otsb = ctx.enter_context(tc.tile_pool(name="otsb", bufs=2))
        # PSUM: st pool 2x[128,1024] = 4 banks; bank pool 4x[128,512] = 4 banks
        pst = ctx.enter_context(tc.tile_pool(name="pst", bufs=2, space="PSUM"))
        pbank = ctx.enter_context(tc.tile_pool(name="pbank", bufs=4, space="PSUM"))

        # ---- constants ----
        wqk_sb = const.tile([128, ND, 128], f16, tag="wqk")
        wv_sb = const.tile([128, ND, 64], f16, tag="wv")
        dup_sb = const.tile([128, 128], f16, tag="dup")
        idn_sb = const.tile([64, 64], f16, tag="idn")
        mask_sb = const.tile([128, 4 * 512], f16, tag="mask")
        bias_sb = const.tile([128, 1], f32, tag="bias")
        nc.gpsimd.memset(bias_sb[:], -4.0)

        nc.sync.dma_start(wqk_sb[:, :, 0:64], wq_h.ap().rearrange("(d p) h -> p d h", p=128))
        nc.sync.dma_start(wqk_sb[:, :, 64:128], wk_h.ap().rearrange("(d p) h -> p d h", p=128))
        nc.sync.dma_start(wv_sb[:], wv_h.ap().rearrange("(d p) h -> p d h", p=128))
        nc.sync.dma_start(dup_sb[:], dup_h.ap())
        nc.sync.dma_start(idn_sb[:], idn_h.ap())

        # causal 0/1 mask for the 4 diagonal j-blocks of a q-group:
        # keep 1.0 where col >= p + 128*m (tk <= tq), else 0.
        nc.gpsimd.memset(mask_sb[:], 1.0)
        nc.gpsimd.affine_select(
            out=mask_sb[:].rearrange("p (m c) -> p m c", c=512),
            in_=mask_sb[:].rearrange("p (m c) -> p m c", c=512),
            compare_op=ALU.is_ge,
            fill=0.0,
            base=0,
            pattern=[[-128, 4], [1, 512]],
            channel_multiplier=-1,
        )

        # ---- x load ----
        x_sb = xpool.tile([128, ND, T], f16, tag="x")
        for d in range(ND):
            nc.sync.dma_start(x_sb[:, d, :], xt_h.ap()[d * 128:(d + 1) * 128, :])

        # ---- persistent projection outputs ----
        q2t = proj.tile([128, T], f16, tag="q2t")   # [q; q] (dup rows)
        k2t = proj.tile([128, T], f16, tag="k2t")   # [k; k]
        vt = proj.tile([64, T], f16, tag="vt")      # vT
        v_sb = proj.tile([128, NJ, 65], f16, tag="vsb")  # V natural + ones col

        nc.gpsimd.memset(v_sb[:], 0.0)  # ensures ones-col region defined
        nc.gpsimd.memset(v_sb[:, :, 64:65], 1.0)

        # ---- QKV projections ----
        for nt in range(NT):
            cols = slice(nt * 512, (nt + 1) * 512)
            p_qk = pbank.tile([128, 512], f32, tag="bank")
            for d in range(ND):
                nc.tensor.matmul(
                    p_qk[:], wqk_sb[:, d, :], x_sb[:, d, cols],
                    start=(d == 0), stop=(d == ND - 1),
                )
            qk_sb = qkpool.tile([128, 512], f16, tag="qk")
            nc.vector.tensor_copy(qk_sb[:], p_qk[:])
            # duplication matmuls: [q;q] and [k;k]
            p_q2 = pbank.tile([128, 512], f32, tag="bank")
            nc.tensor.matmul(p_q2[:], dup_sb[0:64, :], qk_sb[0:64, :], start=True, stop=True)
            nc.vector.tensor_copy(q2t[:, cols], p_q2[:])
            p_k2 = pbank.tile([128, 512], f32, tag="bank")
            nc.tensor.matmul(p_k2[:], dup_sb[64:128, :], qk_sb[64:128, :], start=True, stop=True)
            nc.vector.tensor_copy(k2t[:, cols], p_k2[:])
            # v projection
            p_v = pbank.tile([64, 512], f32, tag="bank")
            for d in range(ND):
                nc.tensor.matmul(
                    p_v[:], wv_sb[:, d, :], x_sb[:, d, cols],
                    start=(d == 0), stop=(d == ND - 1),
                )
            nc.vector.tensor_copy(vt[:, cols], p_v[:])

        # ---- vT -> V natural (PE transpose, batches of 4 blocks) ----
        for jq in range(NJ // 4):
            p_tr = pbank.tile([128, 4, 64], f16, tag="bank")
            for m in range(4):
                j = jq * 4 + m
                nc.tensor.transpose(
                    p_tr[:, m, :], vt[:, j * 128:(j + 1) * 128], idn_sb[:]
                )
            nc.vector.tensor_copy(v_sb[:, jq * 4:(jq + 1) * 4, 0:64], p_tr[:])

        # ---- attention ----
        for g in range(NT):
            gcols = slice(g * 512, (g + 1) * 512)
            ot = pbank.tile([65, 512], f32, tag="bank")
            nhq = 2 * (g + 1)  # half-quads (pairs of j-blocks)
            for hq in range(nhq):
                jA, jB = 2 * hq, 2 * hq + 1
                st = pst.tile([128, 1024], f32, tag="st")
                nc.tensor.matmul(
                    st[:, 0:512], k2t[0:64, jA * 128:(jA + 1) * 128],
                    q2t[0:64, gcols], start=True, stop=True,
                )
                nc.tensor.matmul(
                    st[:, 512:1024], k2t[64:128, jB * 128:(jB + 1) * 128],
                    q2t[64:128, gcols], start=True, stop=True,
                )
                pt = ptpool.tile([128, 1024], f16, tag="pt")
                nc.scalar.activation(pt[:], st[:], AF.Exp, bias=bias_sb[:], scale=0.125)
                if hq >= nhq - 2:
                    # diagonal half-quad: apply causal 0/1 mask
                    moff = (hq - (nhq - 2)) * 1024
                    nc.vector.tensor_mul(
                        pt[:], pt[:], mask_sb[:, moff:moff + 1024]
                    )
                for mm in range(2):
                    j = jA + mm
                    nc.tensor.matmul(
                        ot[:], v_sb[:, j, :], pt[:, mm * 512:(mm + 1) * 512],
                        start=(j == 0), stop=(j == 4 * g + 3),
                    )
            o_sb = otsb.tile([65, 512], f32, tag="ot")
            nc.vector.tensor_copy(o_sb[:], ot[:])
            nc.sync.dma_start(out_h.ap()[:, gcols], o_sb[:])

    nc.compile()
    _cache["nc"] = nc
    return nc


def kernel(x, W_q, W_k, W_v):
    from concourse.bass_utils import run_bass_kernel_spmd

    nc = _build()

    wq_t = np.ascontiguousarray(W_q.T).astype(np.float16)
    wk_t = np.ascontiguousarray(W_k.T).astype(np.float16)
    wv_t = np.ascontiguousarray(W_v.T).astype(np.float16)
    eye = np.eye(64, dtype=np.float16)
    dup = np.tile(np.concatenate([eye, eye], axis=1), (2, 1)).astype(np.float16)

    in_maps = []
    for b in range(NCORES):
        in_maps.append({
            "xt": np.ascontiguousarray(x[b].T).astype(np.float16),
            "wq": wq_t, "wk": wk_t, "wv": wv_t,
            "dup": dup, "idn": eye,
        })

    trace = bool(int(os.environ.get("KERNEL_TRACE", "0")))
    res = run_bass_kernel_spmd(
        nc, in_maps, core_ids=list(range(NCORES)), trace=trace,
    )
    _cache["last_result"] = res

    out = np.empty((NCORES, T, H), dtype=np.float32)
    for b in range(NCORES):
        ot = res.results[b]["out"]  # [65, T] f32
        out[b] = (ot[:H] / ot[H:H + 1]).T
    return out


# revision 5
# speedup vs baseline: 1.1126x; 1.1126x over previous
"""Single-head causal attention (B=8, T=4096, D=1024, H=64) on 8 TRN2 cores.

Strategy: data-parallel over batch — core b computes attention for x[b]
independently; no collectives. Per core:

  - Host pre-transposes x[b] -> xT [D, T] fp16 (layout choice; all FLOPs
    stay on device) and passes W_q/W_k/W_v as [D, H] fp16.
  - QK projection as ONE fused M=128 matmul per (d-chunk, n-tile):
    lhsT = [WqT_d | WkT_d] -> psum [qT; kT].
  - q2T/k2T [128, T]: duplicated-row layout (q on partitions 0-63 AND
    64-127) produced via a "duplication matrix" matmul, enabling 2-way
    row-packed S^T matmuls (K=64 each, two concurrent via PE row groups).
  - S^T computed tk-on-partitions: S^T[jblock, g] = kT_j.T @ qT_g, so the
    softmax transpose is never needed: P^T = exp(0.125*S^T - 4) straight
    from PSUM via ScalarE into fp16 SBUF. Fixed bias -4 replaces the
    per-row max (scores are bounded ~12; exact cancellation in the final
    normalization).
  - Causal mask applied post-exp as a 0/1 fp16 multiply on the 4 diagonal
    j-blocks of each q-group only.
  - PV: OT_aug[65, 512] += V_aug_j.T-style matmul with V augmented by a
    ones column, so row 64 accumulates the softmax denominator for free.
  - Output is OT_aug [65, T] f32; host divides by row 64 and transposes.
"""

import os
import sys

sys.path.insert(0, "/opt/trn_rl_repo")

import numpy as np

T, D, H = 4096, 1024, 64
NCORES = 8
ND = D // 128   # 8 contraction chunks
NT = T // 512   # 8 n-tiles / q-groups
NJ = T // 128   # 32 k-blocks

_cache = {}


def _build():
    if "nc" in _cache:
        return _cache["nc"]
    from contextlib import ExitStack

    import concourse.mybir as mybir
    import concourse.tile as tile
    from concourse import bacc

    dt = mybir.dt
    f16, f32 = dt.float16, dt.float32
    AF = mybir.ActivationFunctionType
    ALU = mybir.AluOpType

    nc = bacc.Bacc("TRN2", target_bir_lowering=False, debug=False)

    xt_h = nc.declare_dram_parameter("xt", [D, T], f16, isOutput=False)
    wq_h = nc.declare_dram_parameter("wq", [D, H], f16, isOutput=False)
    wk_h = nc.declare_dram_parameter("wk", [D, H], f16, isOutput=False)
    wv_h = nc.declare_dram_parameter("wv", [D, H], f16, isOutput=False)
    dup_h = nc.declare_dram_parameter("dup", [128, 128], f16, isOutput=False)
    idn_h = nc.declare_dram_parameter("idn", [64, 64], f16, isOutput=False)
    out_h = nc.declare_dram_parameter("out", [H + 1, T], f32, isOutput=True)

    with tile.TileContext(nc) as tc, ExitStack() as ctx:
        const = ctx.enter_context(tc.tile_pool(name="const", bufs=1))
        xpool = ctx.enter_context(tc.tile_pool(name="x", bufs=1))
        proj = ctx.enter_context(tc.tile_pool(name="proj", bufs=1))
        qkpool = ctx.enter_context(tc.tile_pool(name="qksb", bufs=3))
        ptpool = ctx.enter_context(tc.tile_pool(name="pt", bufs=4))
        otsb = ctx.enter_context(tc.tile_pool(name="otsb", bufs=2))
        # PSUM: st pool 2x[128,1024] = 4 banks; bank pool 4x[128,512] = 4 banks
        pst = ctx.enter_context(tc.tile_pool(name="pst", bufs=2, space="PSUM"))
        pbank = ctx.enter_context(tc.tile_pool(name="pbank", bufs=4, space="PSUM"))

        # ---- constants ----
        wqk_sb = const.tile([128, ND, 128], f16, tag="wqk")
        wv_sb = const.tile([128, ND, 64], f16, tag="wv")
        dup_sb = const.tile([128, 128], f16, tag="dup")
        idn_sb = const.tile([64, 64], f16, tag="idn")
        mask_sb = const.tile([128, 4 * 512], f16, tag="mask")
        bias_sb = const.tile([128, 1], f32, tag="bias")
        nc.gpsimd.memset(bias_sb[:], -4.0)

        nc.sync.dma_start(wqk_sb[:, :, 0:64], wq_h.ap().rearrange("(d p) h -> p d h", p=128))
        nc.sync.dma_start(wqk_sb[:, :, 64:128], wk_h.ap().rearrange("(d p) h -> p d h", p=128))
        nc.sync.dma_start(wv_sb[:], wv_h.ap().rearrange("(d p) h -> p d h", p=128))
        nc.sync.dma_start(dup_sb[:], dup_h.ap())
        nc.sync.dma_start(idn_sb[:], idn_h.ap())

        # causal 0/1 mask for the 4 diagonal j-blocks of a q-group:
        # keep 1.0 where col >= p + 128*m (tk <= tq), else 0.
        nc.gpsimd.memset(mask_sb[:], 1.0)
        nc.gpsimd.affine_select(
            out=mask_sb[:].rearrange("p (m c) -> p m c", c=512),
            in_=mask_sb[:].rearrange("p (m c) -> p m c", c=512),
            compare_op=ALU.is_ge,
            fill=0.0,
            base=0,
            pattern=[[-128, 4], [1, 512]],
            channel_multiplier=-1,
        )

        # ---- x load: (d, nt)-granular DMAs so stage 0 starts after ~1MB ----
        x_sb = xpool.tile([128, ND, T], f16, tag="x")
        for nt in range(NT):
            cols = slice(nt * 512, (nt + 1) * 512)
            for d in range(ND):
                nc.sync.dma_start(
                    x_sb[:, d, cols], xt_h.ap()[d * 128:(d + 1) * 128, cols]
                )

        # ---- persistent projection outputs ----
        q2t = proj.tile([128, T], f16, tag="q2t")   # [q; q] (dup rows)
        k2t = proj.tile([128, T], f16, tag="k2t")   # [k; k]
        vt = proj.tile([64, T], f16, tag="vt")      # vT
        v_sb = proj.tile([128, NJ, 65], f16, tag="vsb")  # V natural + ones col

        nc.gpsimd.memset(v_sb[:], 0.0)  # ensures ones-col region defined
        nc.gpsimd.memset(v_sb[:, :, 64:65], 1.0)

        # ---- interleaved stages: project nt=s, then attend g=s ----
        # attention g=s needs q cols of nt=s, k cols nt<=s, V blocks j<=4s+3
        for s in range(NT):
            cols = slice(s * 512, (s + 1) * 512)
            # -- qk projection (fused M=128) --
            p_qk = pbank.tile([128, 512], f32, tag="bank")
            for d in range(ND):
                nc.tensor.matmul(
                    p_qk[:], wqk_sb[:, d, :], x_sb[:, d, cols],
                    start=(d == 0), stop=(d == ND - 1),
                )
            qk_sb = qkpool.tile([128, 512], f16, tag="qk")
            nc.vector.tensor_copy(qk_sb[:], p_qk[:])
            # duplication matmuls: [q;q] and [k;k]
            p_q2 = pbank.tile([128, 512], f32, tag="bank")
            nc.tensor.matmul(p_q2[:], dup_sb[0:64, :], qk_sb[0:64, :], start=True, stop=True)
            nc.vector.tensor_copy(q2t[:, cols], p_q2[:])
            p_k2 = pbank.tile([128, 512], f32, tag="bank")
            nc.tensor.matmul(p_k2[:], dup_sb[64:128, :], qk_sb[64:128, :], start=True, stop=True)
            nc.vector.tensor_copy(k2t[:, cols], p_k2[:])
            # -- v projection --
            p_v = pbank.tile([64, 512], f32, tag="bank")
            for d in range(ND):
                nc.tensor.matmul(
                    p_v[:], wv_sb[:, d, :], x_sb[:, d, cols],
                    start=(d == 0), stop=(d == ND - 1),
                )
            nc.vector.tensor_copy(vt[:, cols], p_v[:])
            # -- vT -> V natural for this stage's 4 j-blocks --
            p_tr = pbank.tile([128, 4, 64], f16, tag="bank")
            for m in range(4):
                j = s * 4 + m
                nc.tensor.transpose(
                    p_tr[:, m, :], vt[:, j * 128:(j + 1) * 128], idn_sb[:]
                )
            nc.vector.tensor_copy(v_sb[:, s * 4:(s + 1) * 4, 0:64], p_tr[:])

            # -- attention for g = s --
            g = s
            gcols = cols
            ot = pbank.tile([65, 512], f32, tag="bank")
            nhq = 2 * (g + 1)  # half-quads (pairs of j-blocks)
            for hq in range(nhq):
                jA, jB = 2 * hq, 2 * hq + 1
                st = pst.tile([128, 1024], f32, tag="st")
                nc.tensor.matmul(
                    st[:, 0:512], k2t[0:64, jA * 128:(jA + 1) * 128],
                    q2t[0:64, gcols], start=True, stop=True,
                )
                nc.tensor.matmul(
                    st[:, 512:1024], k2t[64:128, jB * 128:(jB + 1) * 128],
                    q2t[64:128, gcols], start=True, stop=True,
                )
                pt = ptpool.tile([128, 1024], f16, tag="pt")
                nc.scalar.activation(pt[:], st[:], AF.Exp, bias=bias_sb[:], scale=0.125)
                if hq >= nhq - 2:
                    # diagonal half-quad: apply causal 0/1 mask
                    moff = (hq - (nhq - 2)) * 1024
                    nc.vector.tensor_mul(
                        pt[:], pt[:], mask_sb[:, moff:moff + 1024]
                    )
                for mm in range(2):
                    j = jA + mm
                    nc.tensor.matmul(
                        ot[:], v_sb[:, j, :], pt[:, mm * 512:(mm + 1) * 512],
                        start=(j == 0), stop=(j == 4 * g + 3),
                    )
            o_sb = otsb.tile([65, 512], f32, tag="ot")
            nc.vector.tensor_copy(o_sb[:], ot[:])
            nc.sync.dma_start(out_h.ap()[:, gcols], o_sb[:])

    nc.compile()
    _cache["nc"] = nc
    return nc


def kernel(x, W_q, W_k, W_v):
    from concourse.bass_utils import run_bass_kernel_spmd

    nc = _build()

    wq_t = np.ascontiguousarray(W_q.T).astype(np.float16)
    wk_t = np.ascontiguousarray(W_k.T).astype(np.float16)
    wv_t = np.ascontiguousarray(W_v.T).astype(np.float16)
    eye = np.eye(64, dtype=np.float16)
    dup = np.tile(np.concatenate([eye, eye], axis=1), (2, 1)).astype(np.float16)

    in_maps = []
    for b in range(NCORES):
        in_maps.append({
            "xt": np.ascontiguousarray(x[b].T).astype(np.float16),
            "wq": wq_t, "wk": wk_t, "wv": wv_t,
            "dup": dup, "idn": eye,
        })

    trace = bool(int(os.environ.get("KERNEL_TRACE", "0")))
    if trace:
        try:
            from antenv import axon_hooks  # noqa: F401
        except ImportError:
            trace = False  # profiling shim not installed; run untraced
    res = run_bass_kernel_spmd(
        nc, in_maps, core_ids=list(range(NCORES)), trace=trace,
    )
    _cache["last_result"] = res

    out = np.empty((NCORES, T, H), dtype=np.float32)
    for b in range(NCORES):
        ot = res.results[b]["out"]  # [65, T] f32
        out[b] = (ot[:H] / ot[H:H + 1]).T
    return out
